# revision 32
# baseline (speedup 1.0000x reference)
"""Causal self-attention (B=4, T=2048, C=1024, 16 heads) on 8 trn2 NeuronCores.

Sharding: core c handles batch b = c//2 and head-group hg = c%2 (8 of 16 heads).
Each core computes QKV projection for its heads, causal attention, and a partial
output projection (row-sharded W_proj); the host sums the two partials per batch
and adds b_proj.

v3 design notes (from NTFF traces of the f32r baseline 698us and v2 511us):
 - TRN2 throttles the PE to 50% duty for matmuls whose row (contraction) tile
   is <= 64 (trace: throttle_activity_1 util_limit=0.5 active exactly over
   phase 2, ham k=4 vs k=8).  All phase-2 matmuls are therefore built as
   K=128: each head's k-features live in a [128, T] tile with the other
   head's 64 partitions zeroed, so the moving operand is the full 128-row q
   tile and the zero stationary rows cancel the wrong head.  The K=1 bias /
   rowsum-broadcast matmuls use zero-padded [128, 512] moving tiles the same
   way.
 - All PE matmuls run on bf16 operands (same 1 cyc/row as f32r at N>=256, no
   4x penalty at N<256, cheaper LDWEIGHTS, half DMA/SBUF traffic).
 - Phase 2 is q-stripe-major (qtr) with kt inner; the AV accumulator (py)
   lives for one stripe only and rotates over 3 PSUM banks, so the softmax
   normalization chain (sum -> broadcast -> reciprocal -> mul) never blocks
   the PE.
 - Scores for two consecutive kt blocks share one [128, 1024] PSUM tile and
   a single exp ACTIVATE, halving the Scalar engine's per-call overheads
   (Scalar exp is the secondary bottleneck after the PE).
 - Scores are computed transposed (S^T[k,q]) so AV contracts over k on the
   partition dim; a ones-column appended to V yields row sums via the PE.
 - Causal masking: exact triangular extents per (kt, stripe) plus a
   -1e30 * strict-upper-triangle matmul accumulate on the 128x128 diag block.
 - Softmax skips max-subtraction: scores/8 are ~N(0,1), exp is safe in fp32.
"""
import numpy as np

T = 2048          # tokens per batch element
C = 1024          # embed dim
H = 8             # heads per core
D = 64            # head dim
CC = 8            # contraction chunks (C / 128)
TT = 16           # token tiles (T / 128)

_CACHE = {}


def _build_nc(repeat=1, phases=(1, 2, 3)):
    from concourse import bacc
    import concourse.mybir as mybir
    import concourse.tile as tile

    f32 = mybir.dt.float32
    f32r = mybir.dt.float32r
    bf16 = mybir.dt.bfloat16
    EXP = mybir.ActivationFunctionType.Exp

    nc = bacc.Bacc("TRN2", num_devices=8, debug=False)

    xt_d = nc.dram_tensor("xt", [C, T], bf16, kind="ExternalInput")
    wqkv_d = nc.dram_tensor("wqkv", [C, 1536], bf16, kind="ExternalInput")
    bqk_d = nc.dram_tensor("bqk", [128, 8], f32, kind="ExternalInput")
    bv_d = nc.dram_tensor("bv", [1, 512], bf16, kind="ExternalInput")
    wproj_d = nc.dram_tensor("wproj", [512, C], bf16, kind="ExternalInput")
    # masku = inclusive upper-triangular ones; e is [k rows, q cols], keep q >= k
    masku_d = nc.dram_tensor("masku", [128, 128], bf16, kind="ExternalInput")
    ones_d = nc.dram_tensor("ones", [128, 128], bf16, kind="ExternalInput")
    out_d = nc.dram_tensor("out", [T, C], f32, kind="ExternalOutput")

    with tile.TileContext(nc) as tc:
      for _rep in range(repeat):
        with tc.tile_pool(name="persist", bufs=1) as pp:
            # persistent SBUF (bf16): q^T [feat, T]; per-head zero-padded k^T;
            # v [t, 8*(64+1)]; wproj; yn
            q_sb = [pp.tile([128, T], bf16, tag=f"q{f}", name=f"q{f}") for f in range(4)]
            # kz_sb[2*fq]   : head 2fq   k^T in rows 0:64,   rows 64:128 zero
            # kz_sb[2*fq+1] : head 2fq+1 k^T in rows 64:128, rows 0:64  zero
            kz_sb = [pp.tile([128, T], bf16, tag=f"kz{h}", name=f"kz{h}") for h in range(8)]
            v_sb = [pp.tile([128, H * 65], bf16, tag=f"v{t}", name=f"v{t}") for t in range(TT)]
            wp_sb = [pp.tile([128, C], bf16, tag=f"wp{i}", name=f"wp{i}") for i in range(4)]
            yn_sb = [pp.tile([128, T], bf16, tag=f"yn{i}", name=f"yn{i}") for i in range(4)]
            masku_sb = pp.tile([128, 128], bf16, tag="masku")
            ones_sb = pp.tile([128, 128], bf16, tag="ones")
            bqk_sb = pp.tile([128, 8], f32, tag="bqk")
            # zero-padded moving tile for the K=1-style V-bias matmul
            bv_pad = pp.tile([128, 512], bf16, tag="bvp")

            nc.sync.dma_start(masku_sb[:], masku_d[:])
            nc.sync.dma_start(ones_sb[:], ones_d[:])
            nc.sync.dma_start(bqk_sb[:], bqk_d[:])
            nc.vector.memset(bv_pad[:], 0.0)
            nc.sync.dma_start(bv_pad[0:1, :], bv_d[:])
            for h in range(8):
                z = slice(64, 128) if h % 2 == 0 else slice(0, 64)
                nc.vector.memset(kz_sb[h][z, :], 0.0)

            # ---------------- Phase 1: QKV projection ----------------
            if 1 in phases:
             with (
                tc.tile_pool(name="ph1", bufs=1) as p1,
                tc.tile_pool(name="ps1", bufs=6, space="PSUM") as ps1,
            ):
                x_sb = [p1.tile([128, T], bf16, tag=f"x{c}", name=f"x{c}") for c in range(CC)]
                w_sb = [p1.tile([128, 1536], bf16, tag=f"w{c}", name=f"w{c}") for c in range(CC)]
                for c in range(CC):
                    nc.sync.dma_start(x_sb[c][:], xt_d[c * 128:(c + 1) * 128, :])
                    nc.sync.dma_start(w_sb[c][:], wqkv_d[c * 128:(c + 1) * 128, :])
                for i in range(4):
                    nc.sync.dma_start(wp_sb[i][:], wproj_d[i * 128:(i + 1) * 128, :])
                for t in range(TT):
                    # ones column at position 64 of each head's 65-wide V block
                    nc.vector.tensor_copy(
                        v_sb[t][:].rearrange("p (h e) -> p h e", e=65)[:, :, 64:65],
                        ones_sb[:, 0:8].rearrange("p (h e) -> p h e", e=1),
                    )

                # q/k features; feature tile f covers heads 2(f%4), 2(f%4)+1
                # (f < 4: q, f >= 4: k).  k features first so attention can
                # start as early as possible.
                for f in (4, 5, 6, 7, 0, 1, 2, 3):
                    pq = [ps1.tile([128, 512], f32, tag="pq", name=f"pq{f}_{r}")
                          for r in range(4)]
                    for c in range(CC):
                        for r in range(4):
                            nc.tensor.matmul(
                                pq[r][:], w_sb[c][:, f * 128:(f + 1) * 128],
                                x_sb[c][:, r * 512:(r + 1) * 512],
                                start=(c == 0), stop=(c == CC - 1),
                            )
                    for r in range(4):
                        sl = slice(r * 512, (r + 1) * 512)
                        if f < 4:
                            nc.vector.tensor_scalar_add(
                                q_sb[f][:, sl], pq[r][:], bqk_sb[:, f:f + 1])
                        else:
                            fq = f - 4
                            nc.vector.tensor_scalar_add(
                                kz_sb[2 * fq][0:64, sl], pq[r][0:64, :],
                                bqk_sb[0:64, f:f + 1])
                            nc.vector.tensor_scalar_add(
                                kz_sb[2 * fq + 1][64:128, sl], pq[r][64:128, :],
                                bqk_sb[64:128, f:f + 1])
                # v -> v[t, f] (t on partitions), bias via ones x bv_pad
                for tg in range(TT):
                    pv = ps1.tile([128, 512], f32, tag="pq", name=f"pv{tg}")
                    for c in range(CC):
                        nc.tensor.matmul(
                            pv[:], x_sb[c][:, tg * 128:(tg + 1) * 128],
                            w_sb[c][:, 1024:1536],
                            start=(c == 0), stop=False,
                        )
                    nc.tensor.matmul(
                        pv[:], ones_sb[:], bv_pad[:], start=False, stop=True
                    )
                    nc.vector.tensor_copy(
                        v_sb[tg][:].rearrange("p (h e) -> p h e", e=65)[:, :, 0:64],
                        pv[:].rearrange("p (h e) -> p h e", e=64),
                    )

            # ---------------- Phase 2: causal attention ----------------
            if 2 in phases:
             with (
                tc.tile_pool(name="ph2", bufs=2) as p2,
                tc.tile_pool(name="epool", bufs=3) as ep,
                tc.tile_pool(name="pss", bufs=3, space="PSUM") as pss,
                tc.tile_pool(name="psy", bufs=2, space="PSUM") as psy,
            ):
                # flat unit list: (h, qtr, kp); kt pair (2kp, 2kp+1)
                units = [(h, qtr, kp)
                         for h in range(H)
                         for qtr in range(4)
                         for kp in range(2 * qtr + 2)]
                py_t = {}

                def emit_scores(u):
                    h, qtr, kp = u
                    q0 = qtr * 512
                    # two kt blocks (2kp, 2kp+1) share one PSUM tile
                    ps = pss.tile([128, 1024], f32, tag="s",
                                  name=f"s{h}_{qtr}_{kp}")
                    aa = []
                    for half in range(2):
                        kt = 2 * kp + half
                        a = max(kt * 128, q0)   # first causal q
                        off = half * 512 - q0
                        nc.tensor.matmul(
                            ps[:, a + off:q0 + 512 + off],
                            kz_sb[h][:, kt * 128:(kt + 1) * 128],
                            q_sb[h // 2][:, a:q0 + 512],
                            start=True, stop=True,
                        )
                        aa.append(a)
                    return ps, aa

                def emit_exp(u, ps, aa):
                    h, qtr, kp = u
                    q0 = qtr * 512
                    e = ep.tile([128, 1024], bf16, tag="e",
                                name=f"e{h}_{qtr}_{kp}")
                    lo = aa[0] - q0
                    nc.scalar.activation(
                        e[:, lo:1024], ps[:, lo:1024], EXP, scale=0.125,
                    )
                    # causal mask inside the two diagonal 128-blocks: zero the
                    # upper-strict-triangle entries of e on the (idle) GpSimd
                    for half in range(2):
                        kt = 2 * kp + half
                        if kt * 128 >= q0:      # diag block for this stripe
                            a = max(kt * 128, q0)
                            off = half * 512 - q0
                            sl = slice(a + off, a + off + 128)
                            nc.gpsimd.tensor_mul(e[:, sl], e[:, sl], masku_sb[:])
                    return e

                def emit_av(u, e, aa):
                    h, qtr, kp = u
                    q0 = qtr * 512
                    ktmax = 4 * qtr + 3
                    if (h, qtr) not in py_t:
                        py_t[h, qtr] = psy.tile([65, 512], f32, tag="py",
                                                name=f"py{h}_{qtr}")
                    py = py_t[h, qtr]
                    for half in range(2):
                        kt = 2 * kp + half
                        a = aa[half]
                        off = half * 512 - q0
                        nc.tensor.matmul(
                            py[:, a - q0:512], v_sb[kt][:, h * 65:(h + 1) * 65],
                            e[:, a + off:q0 + 512 + off],
                            start=(kt == 0), stop=(kt == ktmax),
                        )

                def emit_norm(u):
                    # normalization: yn = y / rowsum.  Row sums are copied out
                    # on the Scalar engine, broadcast to 64 partitions on the
                    # GpSimd, reciprocated and multiplied on the DVE -- no PE
                    # involvement, so the AV pipeline never waits on it.
                    h, qtr, kp = u
                    q0 = qtr * 512
                    py = py_t.pop((h, qtr))
                    sm = p2.tile([1, 512], f32, tag="sm")
                    rb = p2.tile([64, 512], f32, tag="rb")
                    ri = p2.tile([64, 512], f32, tag="ri")
                    nc.scalar.copy(sm[:], py[64:65, :])
                    nc.gpsimd.partition_broadcast(rb[:], sm[:], channels=64)
                    nc.vector.reciprocal_approx_fast(ri[:], rb[:])
                    nc.vector.tensor_mul(
                        yn_sb[h // 2][(h % 2) * 64:(h % 2) * 64 + 64,
                                      q0:q0 + 512],
                        py[0:64, :], ri[:],
                    )

                # global 2-ahead software pipeline: scores of units i+1 and
                # i+2 are emitted before AV of unit i, across stripe and head
                # boundaries, so the PE never head-of-line blocks on exp
                pend = []
                def retire(p):
                    emit_av(*p)
                    hp, qp, kpp = p[0]
                    if kpp == 2 * qp + 1:       # last pair of its stripe
                        emit_norm(p[0])
                for u in units:
                    ps, aa = emit_scores(u)
                    pend.append((u, emit_exp(u, ps, aa), aa))
                    if len(pend) > 2:
                        retire(pend.pop(0))
                for p in pend:
                    retire(p)

            # ---------------- Phase 3: output projection (partial) ----------------
            if 3 in phases:
             with (
                tc.tile_pool(name="ph3", bufs=2) as p3,
                tc.tile_pool(name="ps3", bufs=3, space="PSUM") as ps3,
            ):
                for tt in range(TT):
                    po = ps3.tile([128, C], f32, tag="po")
                    for fc in range(4):
                        for n in range(2):
                            nc.tensor.matmul(
                                po[:, n * 512:(n + 1) * 512],
                                yn_sb[fc][:, tt * 128:(tt + 1) * 128],
                                wp_sb[fc][:, n * 512:(n + 1) * 512],
                                start=(fc == 0), stop=(fc == 3),
                            )
                    ob = p3.tile([128, C], f32, tag="ob")
                    nc.vector.tensor_copy(ob[:, 0:512], po[:, 0:512])
                    nc.scalar.copy(ob[:, 512:1024], po[:, 512:1024])
                    nc.sync.dma_start(out_d[tt * 128:(tt + 1) * 128, :], ob[:])

    nc.compile()
    return nc


def _get_nc():
    if "nc" not in _CACHE:
        _CACHE["nc"] = _build_nc()
    return _CACHE["nc"]


def build_in_maps(x, W_attn, b_attn, W_proj):
    import ml_dtypes
    bf = ml_dtypes.bfloat16

    x = np.asarray(x, dtype=np.float32)
    W_attn = np.asarray(W_attn, dtype=np.float32)
    b_attn = np.asarray(b_attn, dtype=np.float32)
    W_proj = np.asarray(W_proj, dtype=np.float32)

    masku = np.ascontiguousarray(np.triu(np.ones((128, 128), np.float32)).astype(bf))
    ones = np.ones((128, 128), bf)
    xts = [np.ascontiguousarray(x[b].T.astype(bf)) for b in range(4)]

    in_maps = []
    for c in range(8):
        b, hg = divmod(c, 2)
        s = hg * 512
        wqkv = np.ascontiguousarray(np.concatenate(
            [W_attn[:, s:s + 512],
             W_attn[:, 1024 + s:1024 + s + 512],
             W_attn[:, 2048 + s:2048 + s + 512]], axis=1).astype(bf))
        bqk = np.ascontiguousarray(
            np.concatenate([b_attn[s:s + 512], b_attn[1024 + s:1024 + s + 512]])
            .reshape(8, 128).T)
        bv = np.ascontiguousarray(b_attn[2048 + s:2048 + s + 512]
                                  .reshape(1, 512).astype(bf))
        wproj = np.ascontiguousarray(W_proj[s:s + 512, :].astype(bf))
        in_maps.append({"xt": xts[b], "wqkv": wqkv, "bqk": bqk, "bv": bv,
                        "wproj": wproj, "masku": masku, "ones": ones})
    return in_maps


def kernel(x, W_attn, b_attn, W_proj, b_proj):
    from concourse.bass_utils import run_bass_kernel_spmd

    b_proj = np.asarray(b_proj, dtype=np.float32)
    nc = _get_nc()
    in_maps = build_in_maps(x, W_attn, b_attn, W_proj)
    res = run_bass_kernel_spmd(nc, in_maps, core_ids=list(range(8)))
    y = np.empty((4, T, C), np.float32)
    for b in range(4):
        y[b] = res.results[2 * b]["out"] + res.results[2 * b + 1]["out"] + b_proj
    return y
